# revision 23
# baseline (speedup 1.0000x reference)
"""AttnPool Trainium2 kernel (nn_AttnPool_73100343378373), v2.

Math (algebraically identical to the reference):
    qw     = q @ w                      (H, D)   [qw trick: the big keys
             GEMM x@w.T collapses into this tiny precompute]
    scores = qw @ x.T   per batch       (H, L)
    attn   = softmax(scores + mask_bias, axis=L)
    out    = attn @ x                   (B, H*D)

Distribution: data-parallel over batch, 2 batches per core, q/w replicated.

Precision scheme (validated against the reference in fp64/numpy):
  - qw is computed on device in fp32 PSUM from bf16 planes:
    qw = qh@wh + ql@wh + qh@wl (dropped ql@wl term is ~1e-4 relative).
  - qw is split into bf16 planes qw = qwh + qwl.
  - NTERM=2: scores = qwh@xh.T + qwl@xh.T   (x_lo dropped entirely;
    rel err vs reference 1.21e-2, gate is 2e-2).
  - NTERM=3: adds qwh@xl.T (rel err 3.0e-3) -- fallback scheme.
  - pooling uses the hi plane only (error contribution ~1e-4: attn is a
    weighted average so the dropped lo plane is bounded by max|x_lo|).

Schedule notes:
  - x.T tiles are produced by PE transposes (contraction over D forces a
    transposed operand; nothing else on the chip can do this fast).
  - transposes and score matmuls are interleaved per 2-chunk pair so the
    PE never idles >1us (keeps the HAM clock gate at K=8/8 = 2.4 GHz).
  - score matmuls use stacked [qwh|qwl] 16-col weights (one stream for
    both terms) at 2 column-strip positions for concurrent streams.
  - the strip reduce is fused with the running max via
    tensor_tensor_reduce; the final pooled scale is fused into the ACT
    psum->sbuf drain.
"""

import os
from contextlib import ExitStack

import numpy as np

B, L, D, H = 16, 4096, 1024, 8
NCORES = 8
BPC = B // NCORES  # batches per core
NG = 8  # L-groups per batch
GL = L // NG  # rows per group = 512
NT = L // 128  # 128-row L-tiles per batch = 32
DC = D // 128  # 128-wide D chunks = 8

NTERM = int(os.environ.get("ATTNPOOL_NTERM", "2"))  # 2 | 3

VARIANT = {
    "xg_bufs": 10,
    "xt_bufs": 3,
    "w_bufs": 4,
    "pst_bufs": 4,
    "nstrip": 2,      # score/qw strip positions (1 = single at (0,0))
    "use_ttr": True,  # tensor_tensor_reduce for final add+max
    "act_scale": True,  # fused ACT copy-with-scale for pooled
    "bitcast": True,  # f32-bitcast psum->sbuf copies
}
for _k, _v in os.environ.items():
    if _k.startswith("ATTNPOOL_V_"):
        _name = _k[len("ATTNPOOL_V_"):].lower()
        VARIANT[_name] = int(_v) if _v.lstrip("-").isdigit() else _v


_CACHE: dict = {}
LAST_RESULTS = None  # test harness can read exec_time_ns from here


def _build(masked: bool, nterm: int, variant: dict | None = None):
    import concourse.bass as bass
    import concourse.tile as tile
    from concourse import bacc, mybir
    from concourse.masks import make_identity

    v = dict(VARIANT)
    if variant:
        v.update(variant)
    if nterm == 3:
        v["xg_bufs"] = min(v["xg_bufs"], 8)
        v["xgl_bufs"] = 2
        v["w_bufs"] = 4
        v["xt_bufs"] = min(v["xt_bufs"], 2)
    if masked:
        v["xg_bufs"] = min(v["xg_bufs"], 8)
        v["xgl_bufs"] = 2
        if nterm == 3:
            v["xt_bufs"] = 1
    wide_bufs = 2 if (nterm == 2 and not masked) else 1
    sc_bufs = 1 if nterm == 3 else 2

    f32 = mybir.dt.float32
    bf16 = mybir.dt.bfloat16
    AF = mybir.ActivationFunctionType
    AX = mybir.AxisListType
    ALU = mybir.AluOpType

    nc = bacc.Bacc("TRN2", target_bir_lowering=False, debug=False)

    xp_d = nc.dram_tensor("xh", (BPC, L, D), bf16, kind="ExternalInput").ap()
    if nterm == 3:
        xl_d = nc.dram_tensor("xl", (BPC, L, D), bf16, kind="ExternalInput").ap()
    qt2_d = nc.dram_tensor("qt2", (128, DC * 40), bf16, kind="ExternalInput").ap()
    wh_d = nc.dram_tensor("wh", (D, D), bf16, kind="ExternalInput").ap()
    wl_d = nc.dram_tensor("wl", (D, D), bf16, kind="ExternalInput").ap()
    if masked:
        mb_d = nc.dram_tensor("mb", (BPC, H, L), f32, kind="ExternalInput").ap()
    out_d = nc.dram_tensor("out", (BPC, H, D), f32, kind="ExternalOutput").ap()

    TS = D  # col stride per L-tile in an xg tile
    NEG = np.float32(-1e30)

    with tile.TileContext(nc) as tc, ExitStack() as ctx:
        const = ctx.enter_context(tc.tile_pool(name="const", bufs=1))
        xgp = ctx.enter_context(tc.tile_pool(name="xg", bufs=v["xg_bufs"]))
        xtp = ctx.enter_context(tc.tile_pool(name="xt", bufs=v["xt_bufs"]))
        if nterm == 3:
            xglp = ctx.enter_context(tc.tile_pool(name="xgl", bufs=v["xgl_bufs"]))
        wsp = ctx.enter_context(tc.tile_pool(name="ws", bufs=v["w_bufs"]))
        sbp = ctx.enter_context(tc.tile_pool(name="small", bufs=2))
        pst = ctx.enter_context(tc.tile_pool(name="pst", bufs=v["pst_bufs"], space="PSUM"))
        pss = ctx.enter_context(tc.tile_pool(name="pss", bufs=2, space="PSUM"))
        psp = ctx.enter_context(tc.tile_pool(name="psp", bufs=2, space="PSUM"))

        ident = const.tile([128, 128], bf16, tag="ident")
        make_identity(nc, ident[:])

        qt2 = const.tile([128, DC * 40], bf16, tag="qt2")
        nc.gpsimd.dma_start(qt2[:], qt2_d)

        # ---- stage 0: qw = q @ w in fp32 psum via bf16 3-term trick.
        # Stacked weights [qTh|qTl] (16 cols) stream wh; qTh streams wl.
        # Chunks go to 2 column strips (0, 32) for concurrent streams.
        # Strip s rows: 32s+0:8 accumulate qh@wh + qh@wl, 32s+8:16 ql@wh.
        qwps = [
            pss.tile([128, 512], f32, tag="pss", name=f"qwps{h}") for h in range(2)
        ]
        wtiles = []
        for c in range(DC):
            wht = wsp.tile([128, D], bf16, tag="ws", name="wht")
            nc.gpsimd.dma_start(wht[:], wh_d[128 * c : 128 * (c + 1), :])
            wtiles.append(wht)
        wltiles = []
        for c in range(DC):
            wlt = wsp.tile([128, D], bf16, tag="wsl", name="wlt", bufs=v["w_bufs"])
            nc.gpsimd.dma_start(wlt[:], wl_d[128 * c : 128 * (c + 1), :])
            wltiles.append(wlt)
        for c in range(DC):
            s = c % 2 if v["nstrip"] == 2 else 0
            for h in range(2):
                nc.tensor.matmul(
                    qwps[h][64 * s : 64 * s + 40, :],
                    qt2[:, 40 * c : 40 * c + 40],
                    wtiles[c][:, 512 * h : 512 * (h + 1)],
                    start=(c < v["nstrip"]),
                    stop=False,
                    tile_position=(0, 64 * s),
                    skip_group_check=True,
                )
        for c in range(DC):
            s = c % 2 if v["nstrip"] == 2 else 0
            for h in range(2):
                nc.tensor.matmul(
                    qwps[h][64 * s : 64 * s + 8, :],
                    qt2[:, 40 * c : 40 * c + 8],
                    wltiles[c][:, 512 * h : 512 * (h + 1)],
                    start=False,
                    stop=(c >= DC - v["nstrip"]),
                    tile_position=(0, 64 * s),
                    skip_group_check=True,
                )
        # reduce 4 row-groups -> qw (8, 1024) fp32. GpSimd cannot touch
        # PSUM, so drain all strip rows with one wide ACT copy first.
        qw_sb = const.tile([H, D], f32, tag="qw")
        for h in range(2):
            t1 = sbp.tile([H, 512], f32, tag="g1")
            t2 = sbp.tile([H, 512], f32, tag="g2")
            dst = qw_sb[:, 512 * h : 512 * (h + 1)]
            nc.scalar.copy(t1[:], qwps[h][0:8, :])
            nc.vector.tensor_add(t1[:], t1[:], qwps[h][32:40, :])
            if v["nstrip"] == 2:
                nc.scalar.copy(t2[:], qwps[h][64:72, :])
                nc.vector.tensor_add(t2[:], t2[:], qwps[h][96:104, :])
                nc.vector.tensor_add(dst, t1[:], t2[:])
            else:
                nc.vector.tensor_copy(dst, t1[:])
        # split qw -> bf16 hi/lo planes
        qw_hi = const.tile([H, D], bf16, tag="qw_hi")
        qw_lo = const.tile([H, D], bf16, tag="qw_lo")
        qw_hi32 = const.tile([H, D], f32, tag="qw_hi32")
        nc.vector.tensor_copy(qw_hi[:], qw_sb[:])
        nc.scalar.copy(qw_hi32[:], qw_hi[:])
        nc.vector.tensor_sub(qw_lo[:], qw_sb[:], qw_hi32[:])
        # transpose into stacked qwT2: per chunk 40 cols [hi 8 | pad 24 |
        # lo 8] so the matmul row groups land 32-aligned in PSUM
        qwT2 = const.tile([128, DC * 40], bf16, tag="qwT2")
        qtps = pst.tile([128, DC * 16], bf16, tag="pst", name="qtps")
        nc.gpsimd.memset(qwT2[:], 0.0)
        for c in range(DC):
            nc.tensor.transpose(
                qtps[:, 16 * c : 16 * c + 8],
                qw_hi[:, 128 * c : 128 * (c + 1)],
                ident[0:H, 0:H],
            )
            nc.tensor.transpose(
                qtps[:, 16 * c + 8 : 16 * c + 16],
                qw_lo[:, 128 * c : 128 * (c + 1)],
                ident[0:H, 0:H],
            )
        for c in range(DC):
            nc.vector.tensor_copy(
                qwT2[:, 40 * c : 40 * c + 8].bitcast(f32),
                qtps[:, 16 * c : 16 * c + 8].bitcast(f32),
            )
            nc.scalar.copy(
                qwT2[:, 40 * c + 32 : 40 * c + 40].bitcast(f32),
                qtps[:, 16 * c + 8 : 16 * c + 16].bitcast(f32),
            )

        # ---- main loop over this core's batches
        def copy_bc(idx, dst, src_):
            eng = nc.vector.tensor_copy if idx % 2 == 0 else nc.scalar.copy
            if v["bitcast"]:
                eng(dst.bitcast(f32), src_.bitcast(f32))
            else:
                eng(dst, src_)
        for b in range(BPC):
            if masked:
                mb_sb = sbp.tile([H, L], f32, tag="mb", bufs=1)
                nc.gpsimd.dma_start(mb_sb[:], mb_d[b])

            scoresT = sbp.tile([H, L], f32, tag="scoresT", bufs=sc_bufs)
            pmax = sbp.tile([H, NG], f32, tag="pmax")
            xg_tiles = []
            for g in range(NG):
                xg = xgp.tile([128, 4 * TS], bf16, tag="xg", name="xg")
                nc.sync.dma_start(
                    xg[:].rearrange("p (t d) -> p t d", d=D),
                    xp_d[b, GL * g : GL * (g + 1), :].rearrange(
                        "(t p) d -> p t d", p=128
                    ),
                )
                xg_tiles.append(xg)
                if nterm == 3:
                    xgl = xglp.tile([128, 4 * D], bf16, tag="xgl", name="xgl")
                    nc.sync.dma_start(
                        xgl[:].rearrange("p (t d) -> p t d", d=D),
                        xl_d[b, GL * g : GL * (g + 1), :].rearrange(
                            "(t p) d -> p t d", p=128
                        ),
                    )

                sp = pss.tile([128, 512], f32, tag="pss", name="sp")
                xt = xtp.tile([128, 512 * DC], bf16, tag="xt", name="xt")
                if nterm == 3:
                    xtl = xtp.tile(
                        [128, 512 * DC], bf16, tag="xtl", name="xtl",
                        bufs=v["xt_bufs"],
                    )
                # per 2-chunk pair: 8 transposes -> psum -> sbuf copy ->
                # 2 score matmuls; keeps MMs peppered through the stream.
                for jp in range(DC // 2):
                    ps = pst.tile([128, 1024], bf16, tag="pst", name="xtps")
                    for k in range(2):
                        j = 2 * jp + k
                        for t in range(4):
                            nc.tensor.transpose(
                                ps[:, 512 * k + 128 * t : 512 * k + 128 * (t + 1)],
                                xg[:, TS * t + 128 * j : TS * t + 128 * (j + 1)],
                                ident[:],
                            )
                    dst = xt[:, 1024 * jp : 1024 * (jp + 1)]
                    copy_bc(jp, dst, ps[:])
                    if nterm == 3:
                        psl = pst.tile([128, 1024], bf16, tag="pst", name="xtpsl")
                        for k in range(2):
                            j = 2 * jp + k
                            for t in range(4):
                                nc.tensor.transpose(
                                    psl[:, 512 * k + 128 * t : 512 * k + 128 * (t + 1)],
                                    xgl[:, D * t + 128 * j : D * t + 128 * (j + 1)],
                                    ident[:],
                                )
                        dstl = xtl[:, 1024 * jp : 1024 * (jp + 1)]
                        copy_bc(jp + 1, dstl, psl[:])
                    for k in range(2):
                        j = 2 * jp + k
                        s = j % 2 if v["nstrip"] == 2 else 0
                        nc.tensor.matmul(
                            sp[64 * s : 64 * s + 40, :],
                            qwT2[:, 40 * j : 40 * j + 40],
                            xt[:, 512 * j : 512 * (j + 1)],
                            start=(j < v["nstrip"]),
                            stop=(j >= DC - v["nstrip"]),
                            tile_position=(0, 64 * s),
                            skip_group_check=True,
                        )
                        if nterm == 3:
                            nc.tensor.matmul(
                                sp[64 * s : 64 * s + 8, :],
                                qwT2[:, 40 * j : 40 * j + 8],
                                xtl[:, 512 * j : 512 * (j + 1)],
                                start=False,
                                stop=(j >= DC - v["nstrip"]),
                                tile_position=(0, 64 * s),
                                skip_group_check=True,
                            )

                # reduce row groups {0:8, 8:16, 32:40, 40:48} -> scores
                # slab: one wide ACT drain, then SBUF adds on GPS/DVE with
                # the final add fused with the running max.
                sl = scoresT[:, GL * g : GL * (g + 1)]
                g1 = sbp.tile([H, 512], f32, tag="g1")
                g2 = sbp.tile([H, 512], f32, tag="g2")
                nc.scalar.copy(g1[:], sp[0:8, :])
                nc.vector.tensor_add(g1[:], g1[:], sp[32:40, :])
                if v["nstrip"] == 2:
                    nc.scalar.copy(g2[:], sp[64:72, :])
                    nc.vector.tensor_add(g2[:], g2[:], sp[96:104, :])
                terms = [g1[:], g2[:]] if v["nstrip"] == 2 else [g1[:], None]
                if masked:
                    g3 = sbp.tile([H, 512], f32, tag="g3")
                    if v["nstrip"] == 2:
                        nc.vector.tensor_add(g3[:], g1[:], g2[:])
                    else:
                        nc.vector.tensor_copy(g3[:], g1[:])
                    terms = [g3[:], mb_sb[:, GL * g : GL * (g + 1)]]
                if terms[1] is None:
                    nc.vector.tensor_copy(sl, g1[:])
                    nc.vector.reduce_max(pmax[:, g : g + 1], sl, axis=AX.X)
                elif v["use_ttr"]:
                    nc.vector.tensor_tensor_reduce(
                        sl, terms[0], terms[1], 1.0, float(NEG), ALU.add, ALU.max,
                        accum_out=pmax[:, g : g + 1],
                    )
                else:
                    nc.vector.tensor_add(sl, terms[0], terms[1])
                    nc.vector.reduce_max(pmax[:, g : g + 1], sl, axis=AX.X)

            negmax = sbp.tile([H, 1], f32, tag="negmax")
            nc.vector.reduce_max(negmax[:], pmax[:], axis=AX.X, negate=True)
            u_bf = sbp.tile([H, L], bf16, tag="u_bf", bufs=sc_bufs)
            sums = sbp.tile([H, NG], f32, tag="sums")
            for ch in range(NG):
                nc.scalar.activation(
                    u_bf[:, GL * ch : GL * (ch + 1)],
                    scoresT[:, GL * ch : GL * (ch + 1)],
                    AF.Exp,
                    bias=negmax[:],
                    scale=1.0,
                    accum_out=sums[:, ch : ch + 1],
                )
            stot = sbp.tile([H, 1], f32, tag="stot")
            nc.vector.reduce_sum(stot[:], sums[:], axis=AX.X)
            inv = sbp.tile([H, 1], f32, tag="inv")
            nc.vector.reciprocal(inv[:], stot[:])

            # transpose u -> uT (128 L-part, 8 H per L-tile)
            uT = sbp.tile([128, NT * H], bf16, tag="uT")
            for ib in range(NT // 4):
                ups = pst.tile([128, 32], bf16, tag="pst", name="utps")
                for k in range(4):
                    i = 4 * ib + k
                    nc.tensor.transpose(
                        ups[:, 8 * k : 8 * (k + 1)],
                        u_bf[:, 128 * i : 128 * (i + 1)],
                        ident[0:H, 0:H],
                    )
                copy_bc(ib, uT[:, 32 * ib : 32 * (ib + 1)], ups[:])

            # pooling: pooled += uT.T @ x_hi, strips i%4
            pp = [
                psp.tile([128, 512], f32, tag="psp", name=f"pp{i}") for i in range(2)
            ]
            for i in range(NT):
                g_, t_ = i // 4, i % 4
                s = i % 4
                for hh in range(2):
                    nc.tensor.matmul(
                        pp[hh][32 * s : 32 * s + 8, :],
                        uT[:, 8 * i : 8 * (i + 1)],
                        xg_tiles[g_][:, TS * t_ + 512 * hh : TS * t_ + 512 * (hh + 1)],
                        start=(i < 4),
                        stop=(i >= NT - 4),
                        tile_position=(0, 32 * s),
                        skip_group_check=True,
                    )
            pooled = sbp.tile([H, D], f32, tag="pooled", bufs=1)
            for hh in range(2):
                p1 = sbp.tile([H, 512], f32, tag="p1", bufs=2)
                p2 = sbp.tile([H, 512], f32, tag="p2", bufs=2)
                nc.scalar.copy(p1[:], pp[hh][0:8, :])
                nc.vector.tensor_add(p1[:], p1[:], pp[hh][32:40, :])
                nc.scalar.copy(p2[:], pp[hh][64:72, :])
                nc.vector.tensor_add(p2[:], p2[:], pp[hh][96:104, :])
                nc.vector.tensor_add(p1[:], p1[:], p2[:])
                if v["act_scale"]:
                    # fused scale-by-1/sum on the ACT engine (per-head scale)
                    nc.scalar.mul(pooled[:, 512 * hh : 512 * (hh + 1)], p1[:], inv[:])
                else:
                    nc.vector.tensor_scalar_mul(
                        pooled[:, 512 * hh : 512 * (hh + 1)], p1[:], inv[:]
                    )
            nc.scalar.dma_start(out_d[b], pooled[:])

    nc.compile()
    return nc


def _get_nc(masked: bool, nterm: int):
    key = (masked, nterm)
    if key not in _CACHE:
        _CACHE[key] = _build(masked, nterm)
    return _CACHE[key]


def _split_bf16(x: np.ndarray):
    import ml_dtypes

    hi = x.astype(ml_dtypes.bfloat16)
    lo = (x - hi.astype(np.float32)).astype(ml_dtypes.bfloat16)
    return hi, lo


def make_in_maps(x, kpm, q, w, masked, nterm):
    import ml_dtypes

    bf = ml_dtypes.bfloat16
    qT = np.asarray(q, np.float32).T  # (D, H)
    qTh, qTl = _split_bf16(qT)
    # qt2: (128, DC*40): per chunk c, cols 40c+0:8 = qT_hi[128c+p],
    # 40c+32:40 = qT_lo (32-aligned stacking, zero pad between)
    qt2 = np.zeros((128, DC * 40), dtype=bf)
    for c in range(DC):
        qt2[:, 40 * c : 40 * c + 8] = qTh[128 * c : 128 * (c + 1), :]
        qt2[:, 40 * c + 32 : 40 * c + 40] = qTl[128 * c : 128 * (c + 1), :]
    wh, wl = _split_bf16(np.asarray(w, np.float32))
    xh, xl = _split_bf16(np.asarray(x, np.float32))
    in_maps = []
    for c in range(NCORES):
        m = {
            "qt2": np.ascontiguousarray(qt2),
            "wh": np.ascontiguousarray(wh),
            "wl": np.ascontiguousarray(wl),
        }
        m["xh"] = np.ascontiguousarray(xh[BPC * c : BPC * (c + 1)])
        if nterm == 3:
            m["xl"] = np.ascontiguousarray(xl[BPC * c : BPC * (c + 1)])
        if masked:
            bias = np.where(
                kpm[BPC * c : BPC * (c + 1), None, :], np.float32(-1e30), np.float32(0)
            ).astype(np.float32)
            m["mb"] = np.ascontiguousarray(np.broadcast_to(bias, (BPC, H, L)))
        in_maps.append(m)
    return in_maps


def kernel(**inputs) -> np.ndarray:
    global LAST_RESULTS
    from concourse.bass_utils import run_bass_kernel_spmd

    x = np.asarray(inputs["x"], dtype=np.float32)
    kpm = np.asarray(inputs["kpm"])
    q = np.asarray(inputs["q"], dtype=np.float32)
    w = np.asarray(inputs["w"], dtype=np.float32)

    masked = bool(kpm.any())
    nc = _get_nc(masked, NTERM)
    in_maps = make_in_maps(x, kpm, q, w, masked, NTERM)

    trace = bool(os.environ.get("ATTNPOOL_TRACE"))
    res = run_bass_kernel_spmd(nc, in_maps, list(range(NCORES)), trace=trace)
    LAST_RESULTS = res
    out = np.concatenate(
        [r["out"].reshape(BPC, H * D) for r in res.results], axis=0
    )
    return np.ascontiguousarray(out.astype(np.float32))
